# revision 46
# baseline (speedup 1.0000x reference)
"""DepthDC3x3 dynamic depthwise conv — Trainium2 Bass kernel, 8 NeuronCores.

Per-sample pipeline (data-parallel over batch N=8 -> one sample per core):
  A  = conv3x3(y, w_gk1) + b_gk1                    # [64,128,128]
  K  = conv1x1(A, w_gk2) + b_gk2                    # [576,128,128] -> per-tap
  r  = sum_k K[:,k] * shift_k(x)                    # dynamic depthwise 3x3
  out= conv3x3(r, w_fuse) + b_fuse                  # [64,128,128]

Key design points:
  * Half-split layout: SBUF partitions = (channel 0..63) x (image half),
    half = rows 0..63 / 64..127; all 128 partitions active everywhere.
  * x and y are pre-padded + pre-cast to bf16 on the HOST into the exact
    [128, 66, 130] tile layout (borders baked in) -> no on-chip casts or
    memsets for inputs; chunked DMAs land directly in the big tiles so
    conv1 starts as soon as the first row-chunk arrives.
  * bf16 matmuls, 2x2 tile_position quadrants -> ~4x PE concurrency.
  * conv1's odd (halves-swapped) window is NORMALIZED into a2 via a staging
    tile + swap DMA, so conv2/stage-C/fold see normal parity everywhere:
    no swapped-x copy, no per-stage swap DMAs, no parity bookkeeping.
  * Stage C works on adjacent windows (2p, 2p+1): window+row dims merge into
    one contiguous 8-row span, so each vertical tap-pair (0,3),(1,4),(2,5)
    is ONE fused [128,2048] DVE multiply (custom overlapping AP with even
    strides -> 2x DVE mode); taps 6,7,8 are [128,1024] singles.  Product
    tree is 6 out-of-place adds; the last add writes the rpad rows directly.
  * Eviction load is split ACT/DVE (DVE_EVICT_JPS) to balance the engines;
    folds/conv3 run with >=1-step lag so the in-order PE never stalls on a
    same-step fold (incl. the cross-half halo rows).
  * Output is stored bf16 and upcast on the host.
"""

import threading

import numpy as np
import ml_dtypes

import concourse.bacc as bacc
import concourse.mybir as mybir
from concourse.tile import TileContext

BF = mybir.dt.bfloat16
F32 = mybir.dt.float32

C = 64            # channels
HR = 64           # rows per half
W = 128           # image width
PW = W + 2        # padded width
PR = HR + 2       # padded rows per half
NWIN = 16         # windows per half (4 rows x 128 cols each)
WPX = 512
RW = 4            # rows per window
KS = 3



# input row chunks (padded-row space, 0..66) and their DMA emission order;
# order chosen so the chunks feeding early conv1/stage-C windows land first
CHUNKS = ((0, 18), (18, 34), (34, 50), (50, 66))
Y_ORDER = (0, 3, 1, 2)
X_ORDER = (0, 3, 1, 2)

_lock = threading.Lock()
_cache = {}
UNITS = []  # (first_inst_id, label) markers for trace attribution


def _tap_off(t):
    return divmod(t, KS)


def build_nc():
    nc = bacc.Bacc("TRN2", target_bir_lowering=False, debug=False)

    xpd_d = nc.dram_tensor("xpd", [128, PR, PW], BF, kind="ExternalInput")
    ypd_d = nc.dram_tensor("ypd", [128, PR, PW], BF, kind="ExternalInput")
    w1_d = nc.dram_tensor("w1d", [128, 9 * C], BF, kind="ExternalInput")
    w2_d = nc.dram_tensor("w2d", [128, 9 * C], BF, kind="ExternalInput")
    w3_d = nc.dram_tensor("w3d", [128, 9 * C], BF, kind="ExternalInput")
    b1_d = nc.dram_tensor("b1d", [128, 1], F32, kind="ExternalInput")
    b2_d = nc.dram_tensor("b2d", [128, 9], F32, kind="ExternalInput")
    b3_d = nc.dram_tensor("b3d", [128, 1], F32, kind="ExternalInput")
    out_d = nc.dram_tensor("out", [C, 2 * HR, W], BF, kind="ExternalOutput")

    with TileContext(nc) as tc:
        with (
            tc.tile_pool(name="big", bufs=1) as big,
            tc.tile_pool(name="wpool", bufs=1) as wpool,
            tc.tile_pool(name="kev", bufs=7) as kevp,
            tc.tile_pool(name="prod", bufs=14) as prodp,
            tc.tile_pool(name="accs", bufs=6) as accp,
            tc.tile_pool(name="stag", bufs=3) as stagp,
            tc.tile_pool(name="ps13", bufs=2, space="PSUM") as ps13p,
            tc.tile_pool(name="psC", bufs=4, space="PSUM") as psCp,
        ):
            ypad = big.tile([128, PR, PW], BF)
            # x and its halves-swapped copy live in ONE tile so a single
            # strided AP can span both (the fused 2-tap multiply reads xpad
            # for one tap half and xswp for the other).
            xpad = big.tile([128, PR, PW], BF)
            a2 = big.tile([128, NWIN * WPX], BF)
            rpad = big.tile([128, PR, PW], BF)

            w1t = wpool.tile([128, 9 * C], BF)
            w2t = wpool.tile([128, 9 * C], BF)
            w3t = wpool.tile([128, 9 * C], BF)
            b1t = wpool.tile([128, 1], F32)
            b2t = wpool.tile([128, 9], F32)
            b3t = wpool.tile([128, 1], F32)

            for td, tt in ((w1_d, w1t), (w2_d, w2t), (w3_d, w3t),
                           (b1_d, b1t), (b2_d, b2t), (b3_d, b3t)):
                nc.scalar.dma_start(out=tt[:], in_=td[:])

            # inputs arrive pre-padded/pre-cast; chunked so compute starts
            # after the first chunk. y on the sync HWDGE queue, x behind the
            # weights on the scalar queue, halves-swapped x copies on the
            # vector queue: three independent FIFOs.
            for k in Y_ORDER[:2]:
                r0, r1 = CHUNKS[k]
                nc.sync.dma_start(out=ypad[:, r0:r1, :], in_=ypd_d[:, r0:r1, :])
            for k in X_ORDER:
                r0, r1 = CHUNKS[k]
                nc.scalar.dma_start(out=xpad[:, r0:r1, :], in_=xpd_d[:, r0:r1, :])
            for k in Y_ORDER[2:]:
                r0, r1 = CHUNKS[k]
                nc.scalar.dma_start(out=ypad[:, r0:r1, :], in_=ypd_d[:, r0:r1, :])

            nc.vector.memset(rpad[0:64, 0:1, :], 0.0)
            nc.vector.memset(rpad[64:128, 65:66, :], 0.0)
            nc.vector.memset(rpad[:, :, 0:1], 0.0)
            nc.vector.memset(rpad[:, :, 129:130], 0.0)

            ident = mybir.ActivationFunctionType.Identity

            def conv9(src, wt, psLR, w0):
                """3x3 conv quadrants for adjacent window-pair (w0, w0+1).
                psLR [128,1024]: [:,0:512] win w0 normal; [:,512:1024] win w0+1
                inverted."""
                r0, r1 = RW * w0, RW * (w0 + 1)
                for t in range(9):
                    dy, dx = _tap_off(t)
                    st, sp = (t == 0), (t == 8)
                    lhsL = wt[0:64, t * C:(t + 1) * C]
                    lhsH = wt[64:128, t * C:(t + 1) * C]
                    nc.tensor.matmul(psLR[0:64, 0:512], lhsL,
                                     src[0:64, r0 + dy:r0 + dy + 4, dx:dx + 128],
                                     start=st, stop=sp, tile_position=(0, 0),
                                     skip_group_check=True)
                    nc.tensor.matmul(psLR[64:128, 0:512], lhsH,
                                     src[64:128, r0 + dy:r0 + dy + 4, dx:dx + 128],
                                     start=st, stop=sp, tile_position=(64, 64),
                                     skip_group_check=True)
                    nc.tensor.matmul(psLR[64:128, 512:1024], lhsL,
                                     src[0:64, r1 + dy:r1 + dy + 4, dx:dx + 128],
                                     start=st, stop=sp, tile_position=(0, 64),
                                     skip_group_check=True)
                    nc.tensor.matmul(psLR[0:64, 512:1024], lhsH,
                                     src[64:128, r1 + dy:r1 + dy + 4, dx:dx + 128],
                                     start=st, stop=sp, tile_position=(64, 0),
                                     skip_group_check=True)

            def conv1_pair(p):
                """conv1 for windows (2p, 2p+1); a2 is written NORMALIZED:
                the even window straight from PSUM, the odd (inverted) window
                through a staging tile + halves-swap DMA."""
                w0 = 2 * p
                psLR = ps13p.tile([128, 2 * WPX], F32, tag="ps13", name=f"c1ps{p}")
                conv9(ypad, w1t, psLR, w0)
                nc.scalar.activation(a2[:, w0 * WPX:(w0 + 1) * WPX],
                                     psLR[:, 0:WPX], ident, bias=b1t[:, 0:1])
                a2s = stagp.tile([128, WPX], BF, tag="a2s", name=f"a2s{p}")
                nc.scalar.activation(a2s[:, :], psLR[:, WPX:2 * WPX],
                                     ident, bias=b1t[:, 0:1])
                aw1 = a2[:, (w0 + 1) * WPX:(w0 + 2) * WPX]
                nc.sync.dma_start(out=aw1[0:64], in_=a2s[64:128, :])
                nc.sync.dma_start(out=aw1[64:128], in_=a2s[0:64, :])

            from concourse.ap import AP as _AP
            PRW = PR * PW
            DVE_EVICT_JPS = (0, 2)  # jp pairs whose q3 eviction runs on DVE

            def stage_c(p, pending=None):
                """conv2 + dynamic multiply-sum for adjacent windows (2p, 2p+1).

                Window w0=2p is stored normal in a2, w1=2p+1 inverted.  Taps
                pair as (2jp, 2jp+1); quadrant choice per (tap, window) pins a
                FIXED parity layout in each [128,2048] kev/product tile:
                  [tA-w0 (norm) | tB-w1 (norm) | tB-w0 (inv) | tA-w1 (inv)]
                so whole-tile adds reduce both parities at once, and the fold
                consumes contiguous halves.  Multiplies read x through custom
                APs on xboth (normal quarters from the xpad region, inverted
                quarters from the xswp region).
                """
                w0, w1 = 2 * p, 2 * p + 1
                xb = xpad[:]
                pstride = xb.ap[0][0]
                xoff = xb.offset
                r0 = RW * w0
                prods = []
                singles = []

                def conv2_mm(t, wv, psm):
                    lhs_l = w2t[0:64, t * C:(t + 1) * C]
                    lhs_h = w2t[64:128, t * C:(t + 1) * C]
                    awin = a2[:, wv * WPX:(wv + 1) * WPX]
                    nc.tensor.matmul(psm[0:64, :], lhs_l, awin[0:64, :],
                                     tile_position=(0, 0),
                                     skip_group_check=True)
                    nc.tensor.matmul(psm[64:128, :], lhs_h, awin[64:128, :],
                                     tile_position=(64, 64),
                                     skip_group_check=True)

                # vertical tap pairs (dy+1, same dx -> even AP strides);
                # taps 6,7,8 are singles [w0 | w1]; everything normal parity
                for jp, (ta, tb) in enumerate(((0, 3), (1, 4), (2, 5))):
                    dyA, dxA = _tap_off(ta)
                    kev2 = kevp.tile([128, 4 * WPX], BF, tag="kev",
                                     name=f"kv{p}_{jp}")
                    # quarters: [tA-w0 | tA-w1 | tB-w0 | tB-w1]
                    quarters = ((ta, w0), (ta, w1), (tb, w0), (tb, w1))
                    for q, (t, wv) in enumerate(quarters):
                        psm = psCp.tile([128, WPX], F32, tag="ps2",
                                        name=f"ps{p}_{jp}_{q}")
                        conv2_mm(t, wv, psm)
                        dst = kev2[:, q * WPX:(q + 1) * WPX]
                        if q == 3 and jp in DVE_EVICT_JPS:
                            nc.vector.tensor_scalar_add(
                                out=dst, in0=psm[:, :],
                                scalar1=b2t[:, t:t + 1])
                        else:
                            nc.scalar.activation(dst, psm[:, :], ident,
                                                 bias=b2t[:, t:t + 1])
                    # ONE fused multiply: [p, tap(2), win-row(8), col(128)];
                    # the window and row dims merge (adjacent windows)
                    pr = prodp.tile([128, 4 * WPX], BF, tag="prod",
                                    name=f"pr{p}_{jp}")
                    xap = _AP(xb.tensor,
                              xoff + (r0 + dyA) * PW + dxA,
                              [[pstride, 128], [PW, 2], [PW, 8], [1, 128]])
                    nc.vector.tensor_mul(
                        out=pr[:, :].rearrange("p (t rc c) -> p t rc c",
                                               t=2, rc=8),
                        in0=kev2[:, :].rearrange("p (t rc c) -> p t rc c",
                                                 t=2, rc=8),
                        in1=xap)
                    prods.append(pr)
                    if jp == 0 and pending is not None:
                        pending()
                        pending = None

                # single taps 6,7,8: [w0 | w1]
                for t in (6, 7, 8):
                    dy, dx = _tap_off(t)
                    kevS = kevp.tile([128, 2 * WPX], BF, tag="kev",
                                     name=f"kv{p}_s{t}")
                    for wi, wv in enumerate((w0, w1)):
                        psm = psCp.tile([128, WPX], F32, tag="ps2",
                                        name=f"ps{p}_s{t}_{wi}")
                        conv2_mm(t, wv, psm)
                        nc.scalar.activation(kevS[:, wi * WPX:(wi + 1) * WPX],
                                             psm[:, :], ident,
                                             bias=b2t[:, t:t + 1])
                    prS = prodp.tile([128, 2 * WPX], BF, tag="prod",
                                     name=f"pr{p}_s{t}")
                    xapS = _AP(xb.tensor, xoff + (r0 + dy) * PW + dx,
                               [[pstride, 128], [PW, 8], [1, 128]])
                    nc.vector.tensor_mul(
                        out=prS[:, :].rearrange("p (rc c) -> p rc c", rc=8),
                        in0=kevS[:, :].rearrange("p (rc c) -> p rc c", rc=8),
                        in1=xapS)
                    singles.append(prS)

                # product tree on DVE; all tiles normal parity
                t0 = prodp.tile([128, 4 * WPX], BF, tag="prod", name=f"t0{p}")
                nc.vector.tensor_add(out=t0[:, :], in0=prods[0][:, :],
                                     in1=prods[1][:, :])
                T = prodp.tile([128, 4 * WPX], BF, tag="prod", name=f"T{p}")
                nc.vector.tensor_add(out=T[:, :], in0=t0[:, :],
                                     in1=prods[2][:, :])
                s0 = prodp.tile([128, 2 * WPX], BF, tag="prod", name=f"s0{p}")
                nc.vector.tensor_add(out=s0[:, :], in0=singles[0][:, :],
                                     in1=singles[1][:, :])
                s01 = prodp.tile([128, 2 * WPX], BF, tag="prod", name=f"s1{p}")
                nc.vector.tensor_add(out=s01[:, :], in0=s0[:, :],
                                     in1=singles[2][:, :])

                # fold: T's two tap-halves + singles sum -> rpad rows directly
                rout = rpad[:, 8 * p + 1:8 * p + 9, 1:129]
                u = accp.tile([128, 2 * WPX], BF, tag="accis", name=f"u{p}")
                nc.vector.tensor_add(out=u[:, :], in0=T[:, 0:2 * WPX],
                                     in1=T[:, 2 * WPX:4 * WPX])
                nc.vector.tensor_add(
                    out=rout,
                    in0=u[:, :].rearrange("p (r c) -> p r c", r=8),
                    in1=s01[:, :].rearrange("p (r c) -> p r c", r=8))
                if p == 0:
                    nc.sync.dma_start(out=rpad[0:64, 65:66, :],
                                      in_=rpad[64:128, 1:2, :])
                if p == 7:
                    nc.sync.dma_start(out=rpad[64:128, 0:1, :],
                                      in_=rpad[0:64, 64:65, :])
                return None

            def conv3_pair(p):
                w0 = 2 * p
                psLR = ps13p.tile([128, 2 * WPX], F32, tag="ps13", name=f"c3ps{p}")
                conv9(rpad, w3t, psLR, w0)
                st = stagp.tile([128, 2 * WPX], BF, tag="stag", name=f"st{p}")
                nc.scalar.activation(st[:, :], psLR[:, :], ident,
                                     bias=b3t[:, 0:1])
                ra, rb = RW * w0, RW * (w0 + 1)
                nc.sync.dma_start(out=out_d[:, ra:ra + 4, :], in_=st[0:64, 0:512])
                nc.sync.dma_start(out=out_d[:, HR + ra:HR + ra + 4, :],
                                  in_=st[64:128, 0:512])
                nc.sync.dma_start(out=out_d[:, HR + rb:HR + rb + 4, :],
                                  in_=st[0:64, 512:1024])
                nc.sync.dma_start(out=out_d[:, rb:rb + 4, :],
                                  in_=st[64:128, 512:1024])

            # software pipeline. conv1 pairs cover windows (2p, 2p+1); stage-C
            # processes same-parity window pairs (w, w+2). Lag-1 everywhere:
            # each stage_c's conv1 inputs and each conv3's folds (incl. the
            # cross-half halo rows from folds 0 and 13) are emitted at least
            # one step earlier.
            conv1_sched = {0: 0, 1: 7, 2: 1, 3: 2, 4: 3, 5: 4, 6: 5, 7: 6}
            sc_sched = {1: 0, 2: 7, 3: 1, 4: 2, 5: 3, 6: 4, 7: 5, 8: 6}
            c3_sched = {4: 0, 5: 1, 6: 2, 7: 3, 8: 4, 9: 5, 10: 6, 11: 7}
            def mark(label):
                UNITS.append((nc.next_id(), label))

            pending = None
            for step in range(12):
                if step == 1:
                    # sc(0) first: its conv2 only needs c1(0); emitting
                    # c1(7) after keeps the in-order PE from gating it
                    mark("s1:sc(0)")
                    pending = stage_c(0, pending)
                    mark("s1:c1(7)")
                    conv1_pair(7)
                    continue
                if step in conv1_sched:
                    mark(f"s{step}:c1({conv1_sched[step]})")
                    conv1_pair(conv1_sched[step])
                if step in sc_sched:
                    mark(f"s{step}:sc({sc_sched[step]})")
                    pending = stage_c(sc_sched[step], pending)
                elif pending is not None:
                    mark(f"s{step}:fold")
                    pending()
                    pending = None
                if step in c3_sched:
                    mark(f"s{step}:c3({c3_sched[step]})")
                    conv3_pair(c3_sched[step])
            mark("end")

    nc.compile()
    return nc


def _prep_weights(w_gk1, b_gk1, w_gk2, b_gk2, w_fuse, b_fuse):
    bf = ml_dtypes.bfloat16

    def conv_lhst(wc):
        l = np.empty((128, 9 * C), dtype=bf)
        for t in range(9):
            dy, dx = _tap_off(t)
            m = wc[:, :, dy, dx].T.astype(bf)  # [I, O] lhsT
            l[0:64, t * C:(t + 1) * C] = m
            l[64:128, t * C:(t + 1) * C] = m
        return l

    w1d = conv_lhst(np.asarray(w_gk1))
    w3d = conv_lhst(np.asarray(w_fuse))

    w2 = np.asarray(w_gk2).reshape(C * 9, C)
    w2d = np.empty((128, 9 * C), dtype=bf)
    for t in range(9):
        m = w2[t::9, :].T.astype(bf)
        w2d[0:64, t * C:(t + 1) * C] = m
        w2d[64:128, t * C:(t + 1) * C] = m

    b1 = np.asarray(b_gk1, np.float32)
    b3 = np.asarray(b_fuse, np.float32)
    b1d = np.concatenate([b1, b1]).reshape(128, 1)
    b3d = np.concatenate([b3, b3]).reshape(128, 1)
    b2 = np.asarray(b_gk2, np.float32).reshape(C, 9)
    b2d = np.concatenate([b2, b2], axis=0)
    return w1d, w2d, w3d, b1d, b2d, b3d


def _pad_halves(a):
    """[64,128,128] f32 -> [128, 66, 130] bf16, half-split padded layout.

    partitions 0:64 = channels, image half 0 (padded rows = img rows -1..64);
    partitions 64:128 = channels, half 1 (padded rows = img rows 63..128).
    Borders are zero."""
    t = np.zeros((128, PR, PW), dtype=ml_dtypes.bfloat16)
    t[0:64, 1:66, 1:129] = a[:, 0:65, :]
    t[64:128, 0:65, 1:129] = a[:, 63:128, :]
    return t


def kernel(x, y, w_gk1, b_gk1, w_gk2, b_gk2, w_fuse, b_fuse):
    from concourse.bass_utils import run_bass_kernel_spmd

    with _lock:
        if "nc" not in _cache:
            _cache["nc"] = build_nc()
    nc = _cache["nc"]

    w1d, w2d, w3d, b1d, b2d, b3d = _prep_weights(
        w_gk1, b_gk1, w_gk2, b_gk2, w_fuse, b_fuse)

    x = np.asarray(x, np.float32)
    y = np.asarray(y, np.float32)
    n = x.shape[0]
    assert n == 8, f"expected batch 8, got {n}"
    in_maps = []
    for i in range(n):
        in_maps.append({
            "xpd": _pad_halves(x[i]),
            "ypd": _pad_halves(y[i]),
            "w1d": w1d, "w2d": w2d, "w3d": w3d,
            "b1d": b1d, "b2d": b2d, "b3d": b3d,
        })
    res = run_bass_kernel_spmd(nc, in_maps, core_ids=list(range(n)))
    return np.stack([res.results[i]["out"].astype(np.float32) for i in range(n)],
                    axis=0)
